# revision 22
# baseline (speedup 1.0000x reference)
# Trainium2 Bass kernel for nn_DC_and_CE_loss (CE + Dice + feature-regularization).
#
# Single fused device pass (vs the old 2-pass design). Key ideas:
#
# * std_n (the normalized mean-positive feature direction) only depends on
#   `feature` and `target`, so the host computes it exactly (f64) before
#   launch — this removes the pass-1 -> pass-2 device dependency entirely.
# * The per-voxel channel contractions (dot = f . std_n and ss = sum_c f_c^2)
#   run on the otherwise-idle TensorEngine: the feature shard is shipped in a
#   "stationary" interleaved layout [128 = 16ch x 8slot, 128 vox] so each
#   [128,128] fp8 weight tile + one tiny [128,8] selector matmul produces
#   1024 voxel dots as full-width [128, 512] PSUM tiles (FWL loads fp8
#   weights 4/cycle; no PSUM evacuation needed).
# * 1/||f|| = exp(-0.5 * ln(ss + 1e-24)) on ACT (Rsqrt/Reciprocal are banned;
#   Ln/Exp share one table set with the CE exps -> zero table swaps).
# * All masked sums use shifted-relu / shifted-exp encodings so they run as
#   cheap ACT/DVE ops with f32 accum_out instead of the slow (2.8us)
#   scalar_tensor_tensor+accum chains:
#     sum_pos cos       = sum relu(cos + (pos ? 2 : -1e30)) - 2*cnt_pos
#     sum_easy relu cos = sum relu(cos + (easy ? 0 : -1e30))
#     sum p_k           = sum exp(x_k - lns)
#     sum_k p_k y_k     = e^-16 * sum exp(x_k - lns + (y_k ? 16 : -1e30))
#   and CE uses lns = x0 + ln1p(e^{x1-x0} + e^{x2-x0}) so only ln1p's sum is
#   needed from the device (sum x_t and sum x0 are exact host reductions).
# * GPSIMD does nothing (is_equal there costs 14.5us/tile).
# * feature + feature^2 ship as fp8e4 (halves HBM traffic); the top-250
#   selection is protected by a wide candidate set (8192) re-ranked exactly
#   on host in f64 — validated: worst true-top-250 noisy rank = 427.
#
# Host handles (as in the original baseline): masks/dilation from target,
# the global top-k + final_neg dilation, and the tiny f64 combines.

import numpy as np

B, CF, CLS, S = 2, 16, 3, 128
N_CORES = 8
D_PER_CORE = S // (N_CORES // B)       # 32
NV = D_PER_CORE * S * S                # 524288 voxels per core
NVOX = B * S * S * S                   # 4194304
NT = NV // 1024                        # 512 stationary tiles per core
NR = 8                                 # FR rounds per core
TPR = NT // NR                         # 64 tiles per round
COLS = NV // 128                       # 4096
R = 10
TOP_N = 250
SMOOTH = 1e-5
WEIGHT_CE = 1.0
WEIGHT_DICE = 1.0
FR_WEIGHT = 5.0
SHIFT = 16.0                           # exp-mask shift (e^SHIFT rescaled on host)
NEG_INF = -1e30
POS_SHIFT = 2.0
CAND = 16384

_CACHE = {}
LAST_EXEC_NS = {}


def _pin_act_table(mybir, arch):
    """Steer the act-table chooser to the one set that serves BOTH Exp and
    Ln (natural_log_exp_and_others). The default chooser picks the first
    set per function (exp_and_others / natural_log), which thrashes
    ACT_TABLE_LOAD (1.28us each) on every Ln<->Exp alternation — 15 loads
    per kernel. get_activation_tables is functools.cache'd, so in-place
    mutation of the returned sets is seen by insert_act_table_loads."""
    import concourse.hw_specs as hw_specs
    tables = hw_specs.get_activation_tables(arch)
    both = {mybir.ActivationFunctionType.Exp, mybir.ActivationFunctionType.Ln}
    for name, funcs in tables.items():
        if name != "natural_log_exp_and_others":
            funcs -= both


def build_fused():
    import concourse.bacc as bacc
    import concourse.mybir as mybir
    from concourse.tile import TileContext

    f32 = mybir.dt.float32
    bf16 = mybir.dt.bfloat16
    f8 = mybir.dt.float8e4
    alu = mybir.AluOpType
    act = mybir.ActivationFunctionType

    nc = bacc.Bacc("TRN2", debug=False)
    _pin_act_table(mybir, nc.m.arch)
    feat = nc.dram_tensor("feat", [128, NT * 128], f8, kind="ExternalInput").ap()
    fsq = nc.dram_tensor("fsq", [128, NT * 128], f8, kind="ExternalInput").ap()
    dd = nc.dram_tensor("dd", [2, NV], bf16, kind="ExternalInput").ap()
    y1 = nc.dram_tensor("y1", [NV], bf16, kind="ExternalInput").ap()
    y2 = nc.dram_tensor("y2", [NV], bf16, kind="ExternalInput").ap()
    lea = nc.dram_tensor("lea", [128, COLS], bf16, kind="ExternalInput").ap()
    sel = nc.dram_tensor("sel", [128, 16], bf16, kind="ExternalInput").ap()
    cos = nc.dram_tensor("cos", [128, COLS], bf16, kind="ExternalOutput").ap()
    parts = nc.dram_tensor("parts", [128, 64], f32, kind="ExternalOutput").ap()

    with TileContext(nc) as tc, \
         nc.allow_low_precision(reason="bf16/fp8 chains; all sums accumulate f32"):
        with tc.tile_pool(name="const", bufs=1) as cpool, \
             tc.tile_pool(name="ce", bufs=2) as cepool, \
             tc.tile_pool(name="fp", bufs=4) as fpool, \
             tc.tile_pool(name="qp", bufs=4) as qpool, \
             tc.tile_pool(name="mp", bufs=3) as mpool, \
             tc.tile_pool(name="rp", bufs=3) as rpool, \
             tc.tile_pool(name="ps", bufs=3, space="PSUM") as pspool, \
             tc.tile_pool(name="cp", bufs=1, space="PSUM") as cppool:
            P = cpool.tile([128, 64], f32, tag="P")
            nc.vector.memset(P[:], 0.0)
            selt = cpool.tile([128, 16], bf16, tag="sel")
            nc.sync.dma_start(selt[:], sel[:, :])
            bias24 = cpool.tile([128, 1], f32, tag="bias24")
            nc.vector.memset(bias24[:], 1e-24)
            nhalf = cpool.tile([128, 1], f32, tag="nhalf")
            nc.vector.memset(nhalf[:], -0.5)

            nm1 = cpool.tile([128, 1], f32, tag="nm1")
            nc.vector.memset(nm1[:], -1.0)
            ones1 = cpool.tile([128, 1], bf16, tag="ones1")
            nc.vector.memset(ones1[:], 1.0)

            # software-pipelined round DMAs: round r+1's inputs issue
            # while round r computes (pools bufs=2 keep exactly 2 live).
            rt = [None] * NR

            def dma_round(r):
                # quarter-granularity DMAs: the first matmuls of the round
                # only wait on the first 2048-col quarter, not the full chunk
                fc = fpool.tile([128, TPR * 128], f8, tag="fc")
                qc = qpool.tile([128, TPR * 128], f8, tag="qc")
                Q = TPR * 128 // 4
                base = r * TPR * 128
                for q in range(4):
                    nc.sync.dma_start(fc[:, q * Q:(q + 1) * Q],
                                      feat[:, base + q * Q:base + (q + 1) * Q])
                for q in range(4):
                    nc.sync.dma_start(qc[:, q * Q:(q + 1) * Q],
                                      fsq[:, base + q * Q:base + (q + 1) * Q])
                le = mpool.tile([128, 512], bf16, tag="le")
                nc.sync.dma_start(le[:], lea[:, r * 512:(r + 1) * 512])
                rt[r] = (fc, qc, le)

            dma_round(0)

            # CE inputs: whole tiles, but DMA'd in 1024-col quarters spread
            # across the even rounds so they don't starve the feat stream
            m1 = cepool.tile([128, COLS], bf16, tag="m1")
            m2 = cepool.tile([128, COLS], bf16, tag="m2")
            dt1 = cepool.tile([128, COLS], bf16, tag="dt1")
            dt2 = cepool.tile([128, COLS], bf16, tag="dt2")

            def dma_ce_quarter(k):
                sl = slice(1024 * k, 1024 * (k + 1))
                nc.sync.dma_start(m1[:, sl],
                                  y1[:].rearrange("(p f) -> p f", p=128)[:, sl])
                nc.sync.dma_start(m2[:, sl],
                                  y2[:].rearrange("(p f) -> p f", p=128)[:, sl])
                nc.sync.dma_start(dt1[:, sl],
                                  dd[0, :].rearrange("(p f) -> p f", p=128)[:, sl])
                nc.sync.dma_start(dt2[:, sl],
                                  dd[1, :].rearrange("(p f) -> p f", p=128)[:, sl])
            cps = cppool.tile([128, 16], f32, tag="cps")

            NCE = 4
            CW = COLS // NCE

            def ce_chunk(k):
                """CE/dice partial chunk k over cols [CW*k, CW*(k+1))."""
                sl = slice(CW * k, CW * (k + 1))
                e1 = cepool.tile([128, CW], bf16, tag="e1")
                nc.scalar.activation(e1[:], dt1[:, sl], act.Exp)
                e2 = cepool.tile([128, CW], bf16, tag="e2")
                nc.scalar.activation(e2[:], dt2[:, sl], act.Exp)
                sm = cepool.tile([128, CW], bf16, tag="sm")
                nc.vector.tensor_tensor(out=sm[:], in0=e1[:], in1=e2[:], op=alu.add)
                lr = cepool.tile([128, CW], bf16, tag="lr")
                nc.scalar.activation(lr[:], sm[:], act.Ln, bias=1.0,
                                     accum_out=P[:, 40 + k:41 + k])
                wt = cepool.tile([128, CW], bf16, tag="wt")
                nc.scalar.activation(wt[:], lr[:], act.Exp, scale=nm1[:, 0:1])
                p1 = cepool.tile([128, CW], bf16, tag="p1")
                nc.vector.tensor_tensor(out=p1[:], in0=e1[:], in1=wt[:], op=alu.mult)
                p2 = cepool.tile([128, CW], bf16, tag="p2")
                nc.vector.tensor_tensor(out=p2[:], in0=e2[:], in1=wt[:], op=alu.mult)
                t1 = cepool.tile([128, CW], bf16, tag="t1")
                nc.vector.tensor_tensor(out=t1[:], in0=p1[:], in1=m1[:, sl], op=alu.mult)
                t2 = cepool.tile([128, CW], bf16, tag="t2")
                nc.vector.tensor_tensor(out=t2[:], in0=p2[:], in1=m2[:, sl], op=alu.mult)
                for i, src in enumerate((p1, p2, t1, t2)):
                    for c in range(CW // 128):
                        nc.tensor.matmul(cps[:, 4 * k + i:4 * k + i + 1],
                                         src[:, 128 * c:128 * c + 128],
                                         ones1[:, 0:1],
                                         start=(c == 0), stop=(c == CW // 128 - 1))

            # ---------------- FR rounds ----------------
            dma_ce_quarter(0)
            for r in range(NR):
                if r + 1 < NR:
                    dma_round(r + 1)
                if r in (0, 2, 4) :
                    dma_ce_quarter(r // 2 + 1)
                fc, qc, le = rt[r]

                # ss matmuls first so ACT's Ln overlaps the dot matmuls
                pd = pspool.tile([128, 512], f32, tag="pd")
                ps = pspool.tile([128, 512], f32, tag="ps")
                for t in range(TPR):
                    nc.tensor.matmul(ps[:, 8 * t:8 * t + 8],
                                     qc[:, 128 * t:128 * t + 128],
                                     selt[:, 8:16], start=True, stop=True)
                for t in range(TPR):
                    nc.tensor.matmul(pd[:, 8 * t:8 * t + 8],
                                     fc[:, 128 * t:128 * t + 128],
                                     selt[:, 0:8], start=True, stop=True)

                lnt = rpool.tile([128, 512], f32, tag="lnt")
                nc.scalar.activation(lnt[:], ps[:], act.Ln, bias=bias24[:, 0:1])
                rv = rpool.tile([128, 512], bf16, tag="rv")
                nc.scalar.activation(rv[:], lnt[:], act.Exp, scale=nhalf[:, 0:1])
                co = rpool.tile([128, 512], bf16, tag="co")
                nc.vector.tensor_tensor(out=co[:], in0=pd[:], in1=rv[:], op=alu.mult)
                nc.sync.dma_start(cos[:, r * 512:(r + 1) * 512], co[:])
                sp = rpool.tile([128, 512], bf16, tag="sp")
                nc.vector.tensor_scalar(out=sp[:], in0=m1[:, r * 512:(r + 1) * 512],
                                        scalar1=242.0, scalar2=-240.0,
                                        op0=alu.mult, op1=alu.add)
                a1 = rpool.tile([128, 512], bf16, tag="a1")
                nc.vector.tensor_tensor(out=a1[:], in0=co[:], in1=sp[:], op=alu.add)
                z1 = rpool.tile([128, 512], bf16, tag="z1")
                nc.vector.tensor_scalar(out=z1[:], in0=a1[:], scalar1=0.0,
                                        scalar2=0.0, op0=alu.max, op1=alu.add,
                                        accum_out=P[:, 8 + r:9 + r])
                a2 = rpool.tile([128, 512], bf16, tag="a2")
                nc.vector.tensor_tensor(out=a2[:], in0=co[:], in1=le[:], op=alu.add)
                z2 = rpool.tile([128, 512], bf16, tag="z2")
                nc.vector.tensor_scalar(out=z2[:], in0=a2[:], scalar1=0.0,
                                        scalar2=0.0, op0=alu.max, op1=alu.add,
                                        accum_out=P[:, 16 + r:17 + r])
                if r in (1, 2, 4, 6):
                    ce_chunk((1, 2, 4, 6).index(r))

            nc.vector.tensor_copy(P[:, 24:40], cps[:, 0:16])
            nc.sync.dma_start(parts[:, :], P[:])
    nc.finalize()
    return nc


def _run_spmd(key, build_fn, in_maps):
    import os
    import time
    from concourse.bass_utils import run_bass_kernel_spmd
    if key not in _CACHE:
        _CACHE[key] = build_fn()
    nc = _CACHE[key]
    trace = bool(int(os.environ.get("KERNEL_TRACE", "0")))
    t0 = time.perf_counter()
    res = run_bass_kernel_spmd(nc, in_maps, core_ids=list(range(N_CORES)),
                               trace=trace)
    LAST_EXEC_NS[key] = (res.exec_time_ns, time.perf_counter() - t0)
    return res.results


def _dilate(m):
    """Binary box dilation, radius R, separable along axes 1..3 of (B,D,H,W)."""
    x = m.astype(np.int32)
    for ax in (1, 2, 3):
        c = np.cumsum(x, axis=ax, dtype=np.int32)
        n = x.shape[ax]
        hi = np.take(c, np.minimum(np.arange(n) + R, n - 1), axis=ax)
        lo_idx = np.arange(n) - R - 1
        lo = np.take(c, np.maximum(lo_idx, 0), axis=ax)
        shape = [1, 1, 1, 1]
        shape[ax] = n
        valid = (lo_idx >= 0).astype(np.int32).reshape(shape)
        x = hi - lo * valid
    return x > 0


def _to_cos_layout(flat):
    """[NV] flat -> [128, COLS] matching the PE/PSUM voxel layout.

    v = 65536*r + 1024*tau + 8*m + n  lives at  [m, 512*r + 8*tau + n].
    """
    return np.ascontiguousarray(
        flat.reshape(NR, TPR, 128, 8).transpose(2, 0, 1, 3).reshape(128, COLS))


def _from_cos_layout(arr):
    """[128, COLS] device layout -> [NV] flat."""
    return np.ascontiguousarray(
        arr.reshape(128, NR, TPR, 8).transpose(1, 2, 0, 3)).reshape(NV)


def _to_stationary(fcore):
    """[16, NV] f32 channel-major -> [128, NT*128] interleaved stationary.

    out[8c+j, 128t+m] = f[c, 1024t + 8m + j].
    """
    return np.ascontiguousarray(
        fcore.reshape(CF, NT, 128, 8).transpose(0, 3, 1, 2).reshape(128, NT * 128))


def kernel(feature, net_output, target):
    import ml_dtypes
    bf16 = ml_dtypes.bfloat16
    f8 = ml_dtypes.float8_e4m3
    feature = np.ascontiguousarray(np.asarray(feature, dtype=np.float32))
    net_output = np.ascontiguousarray(np.asarray(net_output, dtype=np.float32))
    t3 = np.asarray(target)[:, 0]                      # (B,D,H,W) int32
    pos = t3 == 1
    neg = t3 == 0
    easy = _dilate(pos) & ~pos

    # ---- host: exact std_n (f64 combine of per-batch f32 BLAS matvecs) ----
    possum = np.zeros(CF, np.float64)
    for b in range(B):
        possum += (feature[b].reshape(CF, -1)
                   @ pos[b].reshape(-1).astype(np.float32)).astype(np.float64)
    cnt_pos = float(pos.sum())
    std = possum / max(cnt_pos, 1.0)
    if cnt_pos <= 0:
        std = np.zeros_like(std)
    stdn = std / max(np.linalg.norm(std), 1e-12)

    # ---- host: exact CE linear terms ----
    netf = net_output.reshape(B, CLS, -1)
    sum_x0 = float(netf[:, 0].sum(dtype=np.float64))
    sum_xt = float(np.take_along_axis(
        netf, t3.reshape(B, 1, -1).astype(np.int64), axis=1).sum(dtype=np.float64))

    # ---- selector: cols 0..7 = std_n block-diag, 8..15 = ones block-diag ----
    selm = np.zeros((128, 16), np.float32)
    for c in range(CF):
        for j in range(8):
            selm[8 * c + j, j] = stdn[c]
            selm[8 * c + j, 8 + j] = 1.0
    selm = selm.astype(bf16)

    in_maps = []
    for ci in range(N_CORES):
        b = ci // (N_CORES // B)
        d0 = (ci % (N_CORES // B)) * D_PER_CORE
        fcore = feature[b, :, d0:d0 + D_PER_CORE].reshape(CF, NV)
        fst = _to_stationary(fcore)
        tsh = t3[b, d0:d0 + D_PER_CORE].reshape(NV)
        psh = pos[b, d0:d0 + D_PER_CORE].reshape(NV)
        esh = easy[b, d0:d0 + D_PER_CORE].reshape(NV)
        nsh = net_output[b, :, d0:d0 + D_PER_CORE].reshape(CLS, NV)
        in_maps.append({
            "feat": fst.astype(f8),
            "fsq": (fst.astype(np.float64) ** 2).astype(f8),
            "dd": np.stack([nsh[1] - nsh[0], nsh[2] - nsh[0]]).astype(bf16),
            "y1": _to_cos_layout((tsh == 1).astype(bf16)).reshape(NV),
            "y2": _to_cos_layout((tsh == 2).astype(bf16)).reshape(NV),
            "lea": _to_cos_layout(np.where(esh, np.float32(0.0),
                                           np.float32(NEG_INF)).astype(bf16)),
            "sel": selm,
        })

    results = _run_spmd("fused", build_fused, in_maps)

    # ---- combine partials (f64) ----
    Psum = np.zeros(64, np.float64)
    cos_full = np.empty((B, D_PER_CORE * (N_CORES // B), S, S), np.float32)
    for ci, res in enumerate(results):
        b = ci // (N_CORES // B)
        d0 = (ci % (N_CORES // B)) * D_PER_CORE
        Psum += res["parts"].astype(np.float64).sum(axis=0)
        cos_full[b, d0:d0 + D_PER_CORE] = _from_cos_layout(
            res["cos"].astype(np.float32)).reshape(D_PER_CORE, S, S)

    sum_p1 = Psum[24:40:4].sum()
    sum_p2 = Psum[25:40:4].sum()
    tp1 = Psum[26:40:4].sum()
    tp2 = Psum[27:40:4].sum()
    sum_lns_rel = Psum[40:44].sum()
    poscos = Psum[8:16].sum() - POS_SHIFT * cnt_pos
    easysum = Psum[16:24].sum()

    ce = -(sum_xt - sum_x0 - sum_lns_rel) / NVOX

    cnt1 = float((t3 == 1).sum())
    cnt2 = float((t3 == 2).sum())
    tp = np.array([0.0, tp1, tp2])
    sump = np.array([0.0, sum_p1, sum_p2])
    cntk = np.array([0.0, cnt1, cnt2])
    fp = sump - tp
    fn = cntk - tp
    dc = (2.0 * tp + SMOOTH) / np.maximum(2.0 * tp + fp + fn + SMOOTH, 1e-8)
    dc_loss = -dc[1:].mean()

    pos_loss = (cnt_pos - poscos) / max(cnt_pos, 1.0) if cnt_pos > 0 else 0.0
    easy_cnt = float(easy.sum())
    mis_loss = easysum / max(easy_cnt, 1.0) if easy_cnt > 0 else 0.0

    # ---- host: global top-250 (wide candidate set, exact f64 re-rank) ----
    sims = np.where(neg, cos_full, np.float32(-1e30)).ravel()
    ci_idx = np.argpartition(sims, sims.size - CAND)[-CAND:]
    ci_idx = ci_idx[sims[ci_idx] > -1e29]
    fmat = np.moveaxis(feature, 1, -1).reshape(-1, CF)
    fc = fmat[ci_idx].astype(np.float64)
    nrm = np.maximum(np.linalg.norm(fc, axis=1), 1e-12)
    exact = (fc @ stdn) / nrm
    order = np.argsort(-exact, kind="stable")[:TOP_N]
    keep = ci_idx[order]
    hi = np.zeros(sims.shape, bool)
    hi[keep] = True
    final_neg = _dilate(hi.reshape(B, S, S, S)) & ~pos
    fn_cnt = float(final_neg.sum())
    if fn_cnt > 0:
        neg_loss = float(
            np.maximum(cos_full[final_neg], 0.0).astype(np.float64).sum()
        ) / max(fn_cnt, 1.0)
    else:
        neg_loss = 0.0

    fr = pos_loss + mis_loss + neg_loss
    total = WEIGHT_CE * ce + WEIGHT_DICE * dc_loss + FR_WEIGHT * fr
    return np.asarray(total, dtype=np.float32)


# revision 23
# speedup vs baseline: 1.0963x; 1.0963x over previous
# Trainium2 Bass kernel for nn_DC_and_CE_loss (CE + Dice + feature-regularization).
#
# Single fused device pass (vs the old 2-pass design). Key ideas:
#
# * std_n (the normalized mean-positive feature direction) only depends on
#   `feature` and `target`, so the host computes it exactly (f64) before
#   launch — this removes the pass-1 -> pass-2 device dependency entirely.
# * The per-voxel channel contractions (dot = f . std_n and ss = sum_c f_c^2)
#   run on the otherwise-idle TensorEngine: the feature shard is shipped in a
#   "stationary" interleaved layout [128 = 16ch x 8slot, 128 vox] so each
#   [128,128] fp8 weight tile + one tiny [128,8] selector matmul produces
#   1024 voxel dots as full-width [128, 512] PSUM tiles (FWL loads fp8
#   weights 4/cycle; no PSUM evacuation needed).
# * 1/||f|| = exp(-0.5 * ln(ss + 1e-24)) on ACT (Rsqrt/Reciprocal are banned;
#   Ln/Exp share one table set with the CE exps -> zero table swaps).
# * All masked sums use shifted-relu / shifted-exp encodings so they run as
#   cheap ACT/DVE ops with f32 accum_out instead of the slow (2.8us)
#   scalar_tensor_tensor+accum chains:
#     sum_pos cos       = sum relu(cos + (pos ? 2 : -1e30)) - 2*cnt_pos
#     sum_easy relu cos = sum relu(cos + (easy ? 0 : -1e30))
#     sum p_k           = sum exp(x_k - lns)
#     sum_k p_k y_k     = e^-16 * sum exp(x_k - lns + (y_k ? 16 : -1e30))
#   and CE uses lns = x0 + ln1p(e^{x1-x0} + e^{x2-x0}) so only ln1p's sum is
#   needed from the device (sum x_t and sum x0 are exact host reductions).
# * GPSIMD does nothing (is_equal there costs 14.5us/tile).
# * feature + feature^2 ship as fp8e4 (halves HBM traffic); the top-250
#   selection is protected by a wide candidate set (8192) re-ranked exactly
#   on host in f64 — validated: worst true-top-250 noisy rank = 427.
#
# Host handles (as in the original baseline): masks/dilation from target,
# the global top-k + final_neg dilation, and the tiny f64 combines.

import numpy as np

B, CF, CLS, S = 2, 16, 3, 128
N_CORES = 8
D_PER_CORE = S // (N_CORES // B)       # 32
NV = D_PER_CORE * S * S                # 524288 voxels per core
NVOX = B * S * S * S                   # 4194304
NT = NV // 1024                        # 512 stationary tiles per core
NR = 8                                 # FR rounds per core
TPR = NT // NR                         # 64 tiles per round
COLS = NV // 128                       # 4096
R = 10
TOP_N = 250
SMOOTH = 1e-5
WEIGHT_CE = 1.0
WEIGHT_DICE = 1.0
FR_WEIGHT = 5.0
SHIFT = 16.0                           # exp-mask shift (e^SHIFT rescaled on host)
NEG_INF = -1e30
POS_SHIFT = 2.0
CAND = 16384

_CACHE = {}
LAST_EXEC_NS = {}


def _pin_act_table(mybir, arch):
    """Steer the act-table chooser to the one set that serves BOTH Exp and
    Ln (natural_log_exp_and_others). The default chooser picks the first
    set per function (exp_and_others / natural_log), which thrashes
    ACT_TABLE_LOAD (1.28us each) on every Ln<->Exp alternation — 15 loads
    per kernel. get_activation_tables is functools.cache'd, so in-place
    mutation of the returned sets is seen by insert_act_table_loads."""
    import concourse.hw_specs as hw_specs
    tables = hw_specs.get_activation_tables(arch)
    both = {mybir.ActivationFunctionType.Exp, mybir.ActivationFunctionType.Ln}
    for name, funcs in tables.items():
        if name != "natural_log_exp_and_others":
            funcs -= both


def build_fused():
    import concourse.bacc as bacc
    import concourse.mybir as mybir
    from concourse.tile import TileContext

    f32 = mybir.dt.float32
    bf16 = mybir.dt.bfloat16
    f8 = mybir.dt.float8e4
    alu = mybir.AluOpType
    act = mybir.ActivationFunctionType

    nc = bacc.Bacc("TRN2", debug=False)
    _pin_act_table(mybir, nc.m.arch)
    feat = nc.dram_tensor("feat", [128, NT * 128], f8, kind="ExternalInput").ap()
    fsq = nc.dram_tensor("fsq", [128, NT * 128], f8, kind="ExternalInput").ap()
    dd = nc.dram_tensor("dd", [2, NV], bf16, kind="ExternalInput").ap()
    y1 = nc.dram_tensor("y1", [NV], bf16, kind="ExternalInput").ap()
    y2 = nc.dram_tensor("y2", [NV], bf16, kind="ExternalInput").ap()
    lea = nc.dram_tensor("lea", [128, COLS], bf16, kind="ExternalInput").ap()
    sel = nc.dram_tensor("sel", [128, 16], bf16, kind="ExternalInput").ap()
    cos = nc.dram_tensor("cos", [128, COLS], bf16, kind="ExternalOutput").ap()
    parts = nc.dram_tensor("parts", [128, 64], f32, kind="ExternalOutput").ap()

    with TileContext(nc) as tc, \
         nc.allow_low_precision(reason="bf16/fp8 chains; all sums accumulate f32"):
        with tc.tile_pool(name="const", bufs=1) as cpool, \
             tc.tile_pool(name="ce", bufs=2) as cepool, \
             tc.tile_pool(name="fp", bufs=3) as fpool, \
             tc.tile_pool(name="qp", bufs=3) as qpool, \
             tc.tile_pool(name="mp", bufs=3) as mpool, \
             tc.tile_pool(name="rp", bufs=3) as rpool, \
             tc.tile_pool(name="ps", bufs=3, space="PSUM") as pspool, \
             tc.tile_pool(name="cp", bufs=1, space="PSUM") as cppool:
            P = cpool.tile([128, 64], f32, tag="P")
            nc.vector.memset(P[:], 0.0)
            selt = cpool.tile([128, 16], bf16, tag="sel")
            nc.sync.dma_start(selt[:], sel[:, :])
            bias24 = cpool.tile([128, 1], f32, tag="bias24")
            nc.vector.memset(bias24[:], 1e-24)
            nhalf = cpool.tile([128, 1], f32, tag="nhalf")
            nc.vector.memset(nhalf[:], -0.5)

            nm1 = cpool.tile([128, 1], f32, tag="nm1")
            nc.vector.memset(nm1[:], -1.0)
            ones1 = cpool.tile([128, 1], bf16, tag="ones1")
            nc.vector.memset(ones1[:], 1.0)

            # software-pipelined round DMAs: round r+1's inputs issue
            # while round r computes (pools bufs=2 keep exactly 2 live).
            rt = [None] * NR

            def dma_round(r):
                # quarter-granularity DMAs: the first matmuls of the round
                # only wait on the first 2048-col quarter, not the full chunk
                fc = fpool.tile([128, TPR * 128], f8, tag="fc")
                qc = qpool.tile([128, TPR * 128], f8, tag="qc")
                Q = TPR * 128 // 4
                base = r * TPR * 128
                for q in range(4):
                    nc.sync.dma_start(fc[:, q * Q:(q + 1) * Q],
                                      feat[:, base + q * Q:base + (q + 1) * Q])
                for q in range(4):
                    nc.sync.dma_start(qc[:, q * Q:(q + 1) * Q],
                                      fsq[:, base + q * Q:base + (q + 1) * Q])
                le = mpool.tile([128, 512], bf16, tag="le")
                nc.sync.dma_start(le[:], lea[:, r * 512:(r + 1) * 512])
                rt[r] = (fc, qc, le)

            dma_round(0)

            # ---------------- CE inputs (whole tiles, 1 MiB each) ----------------
            m1 = cepool.tile([128, COLS], bf16, tag="m1")
            nc.sync.dma_start(m1[:], y1[:].rearrange("(p f) -> p f", p=128))
            m2 = cepool.tile([128, COLS], bf16, tag="m2")
            nc.sync.dma_start(m2[:], y2[:].rearrange("(p f) -> p f", p=128))
            dt1 = cepool.tile([128, COLS], bf16, tag="dt1")
            nc.sync.dma_start(dt1[:], dd[0, :].rearrange("(p f) -> p f", p=128))
            dt2 = cepool.tile([128, COLS], bf16, tag="dt2")
            nc.sync.dma_start(dt2[:], dd[1, :].rearrange("(p f) -> p f", p=128))
            cps = cppool.tile([128, 16], f32, tag="cps")

            NCE = 4
            CW = COLS // NCE

            def ce_chunk(k):
                """CE/dice partial chunk k over cols [CW*k, CW*(k+1))."""
                sl = slice(CW * k, CW * (k + 1))
                e1 = cepool.tile([128, CW], bf16, tag="e1")
                nc.scalar.activation(e1[:], dt1[:, sl], act.Exp)
                e2 = cepool.tile([128, CW], bf16, tag="e2")
                nc.scalar.activation(e2[:], dt2[:, sl], act.Exp)
                sm = cepool.tile([128, CW], bf16, tag="sm")
                nc.vector.tensor_tensor(out=sm[:], in0=e1[:], in1=e2[:], op=alu.add)
                lr = cepool.tile([128, CW], bf16, tag="lr")
                nc.scalar.activation(lr[:], sm[:], act.Ln, bias=1.0,
                                     accum_out=P[:, 40 + k:41 + k])
                wt = cepool.tile([128, CW], bf16, tag="wt")
                nc.scalar.activation(wt[:], lr[:], act.Exp, scale=nm1[:, 0:1])
                p1 = cepool.tile([128, CW], bf16, tag="p1")
                nc.vector.tensor_tensor(out=p1[:], in0=e1[:], in1=wt[:], op=alu.mult)
                p2 = cepool.tile([128, CW], bf16, tag="p2")
                nc.vector.tensor_tensor(out=p2[:], in0=e2[:], in1=wt[:], op=alu.mult)
                t1 = cepool.tile([128, CW], bf16, tag="t1")
                nc.vector.tensor_tensor(out=t1[:], in0=p1[:], in1=m1[:, sl], op=alu.mult)
                t2 = cepool.tile([128, CW], bf16, tag="t2")
                nc.vector.tensor_tensor(out=t2[:], in0=p2[:], in1=m2[:, sl], op=alu.mult)
                for i, src in enumerate((p1, p2, t1, t2)):
                    for c in range(CW // 128):
                        nc.tensor.matmul(cps[:, 4 * k + i:4 * k + i + 1],
                                         src[:, 128 * c:128 * c + 128],
                                         ones1[:, 0:1],
                                         start=(c == 0), stop=(c == CW // 128 - 1))

            # ---------------- FR rounds ----------------
            for r in range(NR):
                if r + 1 < NR:
                    dma_round(r + 1)
                fc, qc, le = rt[r]

                # ss matmuls first so ACT's Ln overlaps the dot matmuls
                pd = pspool.tile([128, 512], f32, tag="pd")
                ps = pspool.tile([128, 512], f32, tag="ps")
                for t in range(TPR):
                    nc.tensor.matmul(ps[:, 8 * t:8 * t + 8],
                                     qc[:, 128 * t:128 * t + 128],
                                     selt[:, 8:16], start=True, stop=True)
                for t in range(TPR):
                    nc.tensor.matmul(pd[:, 8 * t:8 * t + 8],
                                     fc[:, 128 * t:128 * t + 128],
                                     selt[:, 0:8], start=True, stop=True)

                lnt = rpool.tile([128, 512], f32, tag="lnt")
                nc.scalar.activation(lnt[:], ps[:], act.Ln, bias=bias24[:, 0:1])
                rv = rpool.tile([128, 512], bf16, tag="rv")
                nc.scalar.activation(rv[:], lnt[:], act.Exp, scale=nhalf[:, 0:1])
                co = rpool.tile([128, 512], bf16, tag="co")
                nc.vector.tensor_tensor(out=co[:], in0=pd[:], in1=rv[:], op=alu.mult)
                nc.sync.dma_start(cos[:, r * 512:(r + 1) * 512], co[:])
                sp = rpool.tile([128, 512], bf16, tag="sp")
                nc.vector.tensor_scalar(out=sp[:], in0=m1[:, r * 512:(r + 1) * 512],
                                        scalar1=242.0, scalar2=-240.0,
                                        op0=alu.mult, op1=alu.add)
                a1 = rpool.tile([128, 512], bf16, tag="a1")
                nc.vector.tensor_tensor(out=a1[:], in0=co[:], in1=sp[:], op=alu.add)
                z1 = rpool.tile([128, 512], bf16, tag="z1")
                nc.vector.tensor_scalar(out=z1[:], in0=a1[:], scalar1=0.0,
                                        scalar2=0.0, op0=alu.max, op1=alu.add,
                                        accum_out=P[:, 8 + r:9 + r])
                a2 = rpool.tile([128, 512], bf16, tag="a2")
                nc.vector.tensor_tensor(out=a2[:], in0=co[:], in1=le[:], op=alu.add)
                z2 = rpool.tile([128, 512], bf16, tag="z2")
                nc.vector.tensor_scalar(out=z2[:], in0=a2[:], scalar1=0.0,
                                        scalar2=0.0, op0=alu.max, op1=alu.add,
                                        accum_out=P[:, 16 + r:17 + r])
                if r in (1, 2, 4, 6):
                    ce_chunk((1, 2, 4, 6).index(r))

            nc.vector.tensor_copy(P[:, 24:40], cps[:, 0:16])
            nc.sync.dma_start(parts[:, :], P[:])
    nc.finalize()
    return nc


def _run_spmd(key, build_fn, in_maps):
    import os
    import time
    from concourse.bass_utils import run_bass_kernel_spmd
    if key not in _CACHE:
        _CACHE[key] = build_fn()
    nc = _CACHE[key]
    trace = bool(int(os.environ.get("KERNEL_TRACE", "0")))
    t0 = time.perf_counter()
    res = run_bass_kernel_spmd(nc, in_maps, core_ids=list(range(N_CORES)),
                               trace=trace)
    LAST_EXEC_NS[key] = (res.exec_time_ns, time.perf_counter() - t0)
    return res.results


def _dilate(m):
    """Binary box dilation, radius R, separable along axes 1..3 of (B,D,H,W)."""
    x = m.astype(np.int32)
    for ax in (1, 2, 3):
        c = np.cumsum(x, axis=ax, dtype=np.int32)
        n = x.shape[ax]
        hi = np.take(c, np.minimum(np.arange(n) + R, n - 1), axis=ax)
        lo_idx = np.arange(n) - R - 1
        lo = np.take(c, np.maximum(lo_idx, 0), axis=ax)
        shape = [1, 1, 1, 1]
        shape[ax] = n
        valid = (lo_idx >= 0).astype(np.int32).reshape(shape)
        x = hi - lo * valid
    return x > 0


def _to_cos_layout(flat):
    """[NV] flat -> [128, COLS] matching the PE/PSUM voxel layout.

    v = 65536*r + 1024*tau + 8*m + n  lives at  [m, 512*r + 8*tau + n].
    """
    return np.ascontiguousarray(
        flat.reshape(NR, TPR, 128, 8).transpose(2, 0, 1, 3).reshape(128, COLS))


def _from_cos_layout(arr):
    """[128, COLS] device layout -> [NV] flat."""
    return np.ascontiguousarray(
        arr.reshape(128, NR, TPR, 8).transpose(1, 2, 0, 3)).reshape(NV)


def _to_stationary(fcore):
    """[16, NV] f32 channel-major -> [128, NT*128] interleaved stationary.

    out[8c+j, 128t+m] = f[c, 1024t + 8m + j].
    """
    return np.ascontiguousarray(
        fcore.reshape(CF, NT, 128, 8).transpose(0, 3, 1, 2).reshape(128, NT * 128))


def kernel(feature, net_output, target):
    import ml_dtypes
    bf16 = ml_dtypes.bfloat16
    f8 = ml_dtypes.float8_e4m3
    feature = np.ascontiguousarray(np.asarray(feature, dtype=np.float32))
    net_output = np.ascontiguousarray(np.asarray(net_output, dtype=np.float32))
    t3 = np.asarray(target)[:, 0]                      # (B,D,H,W) int32
    pos = t3 == 1
    neg = t3 == 0
    easy = _dilate(pos) & ~pos

    # ---- host: exact std_n (f64 combine of per-batch f32 BLAS matvecs) ----
    possum = np.zeros(CF, np.float64)
    for b in range(B):
        possum += (feature[b].reshape(CF, -1)
                   @ pos[b].reshape(-1).astype(np.float32)).astype(np.float64)
    cnt_pos = float(pos.sum())
    std = possum / max(cnt_pos, 1.0)
    if cnt_pos <= 0:
        std = np.zeros_like(std)
    stdn = std / max(np.linalg.norm(std), 1e-12)

    # ---- host: exact CE linear terms ----
    netf = net_output.reshape(B, CLS, -1)
    sum_x0 = float(netf[:, 0].sum(dtype=np.float64))
    sum_xt = float(np.take_along_axis(
        netf, t3.reshape(B, 1, -1).astype(np.int64), axis=1).sum(dtype=np.float64))

    # ---- selector: cols 0..7 = std_n block-diag, 8..15 = ones block-diag ----
    selm = np.zeros((128, 16), np.float32)
    for c in range(CF):
        for j in range(8):
            selm[8 * c + j, j] = stdn[c]
            selm[8 * c + j, 8 + j] = 1.0
    selm = selm.astype(bf16)

    in_maps = []
    for ci in range(N_CORES):
        b = ci // (N_CORES // B)
        d0 = (ci % (N_CORES // B)) * D_PER_CORE
        fcore = feature[b, :, d0:d0 + D_PER_CORE].reshape(CF, NV)
        fst = _to_stationary(fcore)
        tsh = t3[b, d0:d0 + D_PER_CORE].reshape(NV)
        psh = pos[b, d0:d0 + D_PER_CORE].reshape(NV)
        esh = easy[b, d0:d0 + D_PER_CORE].reshape(NV)
        nsh = net_output[b, :, d0:d0 + D_PER_CORE].reshape(CLS, NV)
        in_maps.append({
            "feat": fst.astype(f8),
            "fsq": (fst.astype(np.float64) ** 2).astype(f8),
            "dd": np.stack([nsh[1] - nsh[0], nsh[2] - nsh[0]]).astype(bf16),
            "y1": _to_cos_layout((tsh == 1).astype(bf16)).reshape(NV),
            "y2": _to_cos_layout((tsh == 2).astype(bf16)).reshape(NV),
            "lea": _to_cos_layout(np.where(esh, np.float32(0.0),
                                           np.float32(NEG_INF)).astype(bf16)),
            "sel": selm,
        })

    results = _run_spmd("fused", build_fused, in_maps)

    # ---- combine partials (f64) ----
    Psum = np.zeros(64, np.float64)
    cos_full = np.empty((B, D_PER_CORE * (N_CORES // B), S, S), np.float32)
    for ci, res in enumerate(results):
        b = ci // (N_CORES // B)
        d0 = (ci % (N_CORES // B)) * D_PER_CORE
        Psum += res["parts"].astype(np.float64).sum(axis=0)
        cos_full[b, d0:d0 + D_PER_CORE] = _from_cos_layout(
            res["cos"].astype(np.float32)).reshape(D_PER_CORE, S, S)

    sum_p1 = Psum[24:40:4].sum()
    sum_p2 = Psum[25:40:4].sum()
    tp1 = Psum[26:40:4].sum()
    tp2 = Psum[27:40:4].sum()
    sum_lns_rel = Psum[40:44].sum()
    poscos = Psum[8:16].sum() - POS_SHIFT * cnt_pos
    easysum = Psum[16:24].sum()

    ce = -(sum_xt - sum_x0 - sum_lns_rel) / NVOX

    cnt1 = float((t3 == 1).sum())
    cnt2 = float((t3 == 2).sum())
    tp = np.array([0.0, tp1, tp2])
    sump = np.array([0.0, sum_p1, sum_p2])
    cntk = np.array([0.0, cnt1, cnt2])
    fp = sump - tp
    fn = cntk - tp
    dc = (2.0 * tp + SMOOTH) / np.maximum(2.0 * tp + fp + fn + SMOOTH, 1e-8)
    dc_loss = -dc[1:].mean()

    pos_loss = (cnt_pos - poscos) / max(cnt_pos, 1.0) if cnt_pos > 0 else 0.0
    easy_cnt = float(easy.sum())
    mis_loss = easysum / max(easy_cnt, 1.0) if easy_cnt > 0 else 0.0

    # ---- host: global top-250 (wide candidate set, exact f64 re-rank) ----
    sims = np.where(neg, cos_full, np.float32(-1e30)).ravel()
    ci_idx = np.argpartition(sims, sims.size - CAND)[-CAND:]
    ci_idx = ci_idx[sims[ci_idx] > -1e29]
    fmat = np.moveaxis(feature, 1, -1).reshape(-1, CF)
    fc = fmat[ci_idx].astype(np.float64)
    nrm = np.maximum(np.linalg.norm(fc, axis=1), 1e-12)
    exact = (fc @ stdn) / nrm
    order = np.argsort(-exact, kind="stable")[:TOP_N]
    keep = ci_idx[order]
    hi = np.zeros(sims.shape, bool)
    hi[keep] = True
    final_neg = _dilate(hi.reshape(B, S, S, S)) & ~pos
    fn_cnt = float(final_neg.sum())
    if fn_cnt > 0:
        neg_loss = float(
            np.maximum(cos_full[final_neg], 0.0).astype(np.float64).sum()
        ) / max(fn_cnt, 1.0)
    else:
        neg_loss = 0.0

    fr = pos_loss + mis_loss + neg_loss
    total = WEIGHT_CE * ce + WEIGHT_DICE * dc_loss + FR_WEIGHT * fr
    return np.asarray(total, dtype=np.float32)


# revision 24
# speedup vs baseline: 1.1215x; 1.0230x over previous
# Trainium2 Bass kernel for nn_DC_and_CE_loss (CE + Dice + feature-regularization).
#
# Single fused device pass (vs the old 2-pass design). Key ideas:
#
# * std_n (the normalized mean-positive feature direction) only depends on
#   `feature` and `target`, so the host computes it exactly (f64) before
#   launch — this removes the pass-1 -> pass-2 device dependency entirely.
# * The per-voxel channel contractions (dot = f . std_n and ss = sum_c f_c^2)
#   run on the otherwise-idle TensorEngine: the feature shard is shipped in a
#   "stationary" interleaved layout [128 = 16ch x 8slot, 128 vox] so each
#   [128,128] fp8 weight tile + one tiny [128,8] selector matmul produces
#   1024 voxel dots as full-width [128, 512] PSUM tiles (FWL loads fp8
#   weights 4/cycle; no PSUM evacuation needed).
# * 1/||f|| = exp(-0.5 * ln(ss + 1e-24)) on ACT (Rsqrt/Reciprocal are banned;
#   Ln/Exp share one table set with the CE exps -> zero table swaps).
# * All masked sums use shifted-relu / shifted-exp encodings so they run as
#   cheap ACT/DVE ops with f32 accum_out instead of the slow (2.8us)
#   scalar_tensor_tensor+accum chains:
#     sum_pos cos       = sum relu(cos + (pos ? 2 : -1e30)) - 2*cnt_pos
#     sum_easy relu cos = sum relu(cos + (easy ? 0 : -1e30))
#     sum p_k           = sum exp(x_k - lns)
#     sum_k p_k y_k     = e^-16 * sum exp(x_k - lns + (y_k ? 16 : -1e30))
#   and CE uses lns = x0 + ln1p(e^{x1-x0} + e^{x2-x0}) so only ln1p's sum is
#   needed from the device (sum x_t and sum x0 are exact host reductions).
# * GPSIMD does nothing (is_equal there costs 14.5us/tile).
# * feature + feature^2 ship as fp8e4 (halves HBM traffic); the top-250
#   selection is protected by a wide candidate set (8192) re-ranked exactly
#   on host in f64 — validated: worst true-top-250 noisy rank = 427.
#
# Host handles (as in the original baseline): masks/dilation from target,
# the global top-k + final_neg dilation, and the tiny f64 combines.

import numpy as np

B, CF, CLS, S = 2, 16, 3, 128
N_CORES = 8
D_PER_CORE = S // (N_CORES // B)       # 32
NV = D_PER_CORE * S * S                # 524288 voxels per core
NVOX = B * S * S * S                   # 4194304
NT = NV // 1024                        # 512 stationary tiles per core
NR = 8                                 # FR rounds per core
TPR = NT // NR                         # 64 tiles per round
COLS = NV // 128                       # 4096
R = 10
TOP_N = 250
SMOOTH = 1e-5
WEIGHT_CE = 1.0
WEIGHT_DICE = 1.0
FR_WEIGHT = 5.0
SHIFT = 16.0                           # exp-mask shift (e^SHIFT rescaled on host)
NEG_INF = -1e30
POS_SHIFT = 2.0
CAND = 16384

_CACHE = {}
LAST_EXEC_NS = {}


def _pin_act_table(mybir, arch):
    """Steer the act-table chooser to the one set that serves BOTH Exp and
    Ln (natural_log_exp_and_others). The default chooser picks the first
    set per function (exp_and_others / natural_log), which thrashes
    ACT_TABLE_LOAD (1.28us each) on every Ln<->Exp alternation — 15 loads
    per kernel. get_activation_tables is functools.cache'd, so in-place
    mutation of the returned sets is seen by insert_act_table_loads."""
    import concourse.hw_specs as hw_specs
    tables = hw_specs.get_activation_tables(arch)
    both = {mybir.ActivationFunctionType.Exp, mybir.ActivationFunctionType.Ln}
    for name, funcs in tables.items():
        if name != "natural_log_exp_and_others":
            funcs -= both


def build_fused():
    import concourse.bacc as bacc
    import concourse.mybir as mybir
    from concourse.tile import TileContext

    f32 = mybir.dt.float32
    bf16 = mybir.dt.bfloat16
    f8 = mybir.dt.float8e4
    alu = mybir.AluOpType
    act = mybir.ActivationFunctionType

    nc = bacc.Bacc("TRN2", debug=False)
    _pin_act_table(mybir, nc.m.arch)
    feat = nc.dram_tensor("feat", [128, NT * 128], f8, kind="ExternalInput").ap()
    fsq = nc.dram_tensor("fsq", [128, NT * 128], f8, kind="ExternalInput").ap()
    dd = nc.dram_tensor("dd", [2, NV], bf16, kind="ExternalInput").ap()
    y1 = nc.dram_tensor("y1", [NV], bf16, kind="ExternalInput").ap()
    y2 = nc.dram_tensor("y2", [NV], bf16, kind="ExternalInput").ap()
    lea = nc.dram_tensor("lea", [128, COLS], bf16, kind="ExternalInput").ap()
    sel = nc.dram_tensor("sel", [128, 16], bf16, kind="ExternalInput").ap()
    cos = nc.dram_tensor("cos", [128, COLS], bf16, kind="ExternalOutput").ap()
    parts = nc.dram_tensor("parts", [128, 64], f32, kind="ExternalOutput").ap()

    with TileContext(nc) as tc, \
         nc.allow_low_precision(reason="bf16/fp8 chains; all sums accumulate f32"):
        with tc.tile_pool(name="const", bufs=1) as cpool, \
             tc.tile_pool(name="ce", bufs=2) as cepool, \
             tc.tile_pool(name="fp", bufs=3) as fpool, \
             tc.tile_pool(name="qp", bufs=3) as qpool, \
             tc.tile_pool(name="mp", bufs=3) as mpool, \
             tc.tile_pool(name="rp", bufs=3) as rpool, \
             tc.tile_pool(name="ps", bufs=3, space="PSUM") as pspool, \
             tc.tile_pool(name="cp", bufs=1, space="PSUM") as cppool:
            P = cpool.tile([128, 64], f32, tag="P")
            nc.vector.memset(P[:], 0.0)
            selt = cpool.tile([128, 16], bf16, tag="sel")
            nc.sync.dma_start(selt[:], sel[:, :])
            bias24 = cpool.tile([128, 1], f32, tag="bias24")
            nc.vector.memset(bias24[:], 1e-24)
            nhalf = cpool.tile([128, 1], f32, tag="nhalf")
            nc.vector.memset(nhalf[:], -0.5)

            nm1 = cpool.tile([128, 1], f32, tag="nm1")
            nc.vector.memset(nm1[:], -1.0)
            ones1 = cpool.tile([128, 1], bf16, tag="ones1")
            nc.vector.memset(ones1[:], 1.0)

            # software-pipelined round DMAs: round r+1's inputs issue
            # while round r computes (pools bufs=2 keep exactly 2 live).
            rt = [None] * NR

            def dma_round(r):
                # quarter-granularity DMAs: the first matmuls of the round
                # only wait on the first 2048-col quarter, not the full chunk
                fc = fpool.tile([128, TPR * 128], f8, tag="fc")
                qc = qpool.tile([128, TPR * 128], f8, tag="qc")
                Q = TPR * 128 // 4
                base = r * TPR * 128
                for q in range(4):
                    nc.sync.dma_start(fc[:, q * Q:(q + 1) * Q],
                                      feat[:, base + q * Q:base + (q + 1) * Q])
                for q in range(4):
                    nc.sync.dma_start(qc[:, q * Q:(q + 1) * Q],
                                      fsq[:, base + q * Q:base + (q + 1) * Q])
                le = mpool.tile([128, 512], bf16, tag="le")
                nc.sync.dma_start(le[:], lea[:, r * 512:(r + 1) * 512])
                rt[r] = (fc, qc, le)

            dma_round(0)

            # ---------------- CE inputs (whole tiles, 1 MiB each) ----------------
            m1 = cepool.tile([128, COLS], bf16, tag="m1")
            nc.sync.dma_start(m1[:], y1[:].rearrange("(p f) -> p f", p=128))
            m2 = cepool.tile([128, COLS], bf16, tag="m2")
            nc.sync.dma_start(m2[:], y2[:].rearrange("(p f) -> p f", p=128))
            dt1 = cepool.tile([128, COLS], bf16, tag="dt1")
            nc.sync.dma_start(dt1[:], dd[0, :].rearrange("(p f) -> p f", p=128))
            dt2 = cepool.tile([128, COLS], bf16, tag="dt2")
            nc.sync.dma_start(dt2[:], dd[1, :].rearrange("(p f) -> p f", p=128))
            cps = cppool.tile([128, 16], f32, tag="cps")

            NCE = 4
            CW = COLS // NCE

            def ce_chunk(k):
                """CE/dice partial chunk k over cols [CW*k, CW*(k+1))."""
                sl = slice(CW * k, CW * (k + 1))
                e1 = cepool.tile([128, CW], bf16, tag="e1")
                nc.scalar.activation(e1[:], dt1[:, sl], act.Exp)
                e2 = cepool.tile([128, CW], bf16, tag="e2")
                nc.scalar.activation(e2[:], dt2[:, sl], act.Exp)
                sm = cepool.tile([128, CW], bf16, tag="sm")
                nc.vector.tensor_tensor(out=sm[:], in0=e1[:], in1=e2[:], op=alu.add)
                lr = cepool.tile([128, CW], bf16, tag="lr")
                nc.scalar.activation(lr[:], sm[:], act.Ln, bias=1.0,
                                     accum_out=P[:, 40 + k:41 + k])
                wt = cepool.tile([128, CW], bf16, tag="wt")
                nc.scalar.activation(wt[:], lr[:], act.Exp, scale=nm1[:, 0:1])
                p1 = cepool.tile([128, CW], bf16, tag="p1")
                nc.vector.tensor_tensor(out=p1[:], in0=e1[:], in1=wt[:], op=alu.mult)
                p2 = cepool.tile([128, CW], bf16, tag="p2")
                nc.vector.tensor_tensor(out=p2[:], in0=e2[:], in1=wt[:], op=alu.mult)
                t1 = cepool.tile([128, CW], bf16, tag="t1")
                nc.vector.tensor_tensor(out=t1[:], in0=p1[:], in1=m1[:, sl], op=alu.mult)
                t2 = cepool.tile([128, CW], bf16, tag="t2")
                nc.vector.tensor_tensor(out=t2[:], in0=p2[:], in1=m2[:, sl], op=alu.mult)
                for i, src in enumerate((p1, p2, t1, t2)):
                    for c in range(CW // 128):
                        nc.tensor.matmul(cps[:, 4 * k + i:4 * k + i + 1],
                                         src[:, 128 * c:128 * c + 128],
                                         ones1[:, 0:1],
                                         start=(c == 0), stop=(c == CW // 128 - 1))

            # ---------------- FR rounds ----------------
            for r in range(NR):
                if r + 1 < NR:
                    dma_round(r + 1)
                fc, qc, le = rt[r]

                # ss matmuls first so ACT's Ln overlaps the dot matmuls
                pd = pspool.tile([128, 512], f32, tag="pd")
                ps = pspool.tile([128, 512], f32, tag="ps")
                for t in range(TPR):
                    nc.tensor.matmul(ps[:, 8 * t:8 * t + 8],
                                     qc[:, 128 * t:128 * t + 128],
                                     selt[:, 8:16], start=True, stop=True)
                for t in range(TPR):
                    nc.tensor.matmul(pd[:, 8 * t:8 * t + 8],
                                     fc[:, 128 * t:128 * t + 128],
                                     selt[:, 0:8], start=True, stop=True)

                lnt = rpool.tile([128, 512], f32, tag="lnt")
                nc.scalar.activation(lnt[:], ps[:], act.Ln, bias=bias24[:, 0:1])
                rv = rpool.tile([128, 512], bf16, tag="rv")
                nc.scalar.activation(rv[:], lnt[:], act.Exp, scale=nhalf[:, 0:1])
                co = rpool.tile([128, 512], bf16, tag="co")
                nc.vector.tensor_tensor(out=co[:], in0=pd[:], in1=rv[:], op=alu.mult)
                nc.sync.dma_start(cos[:, r * 512:(r + 1) * 512], co[:])
                sp = rpool.tile([128, 512], bf16, tag="sp")
                nc.vector.tensor_scalar(out=sp[:], in0=m1[:, r * 512:(r + 1) * 512],
                                        scalar1=242.0, scalar2=-240.0,
                                        op0=alu.mult, op1=alu.add)
                a1 = rpool.tile([128, 512], bf16, tag="a1")
                nc.vector.tensor_tensor(out=a1[:], in0=co[:], in1=sp[:], op=alu.add)
                z1 = rpool.tile([128, 512], bf16, tag="z1")
                nc.vector.tensor_scalar(out=z1[:], in0=a1[:], scalar1=0.0,
                                        scalar2=0.0, op0=alu.max, op1=alu.add,
                                        accum_out=P[:, 8 + r:9 + r])
                a2 = rpool.tile([128, 512], bf16, tag="a2")
                nc.vector.tensor_tensor(out=a2[:], in0=co[:], in1=le[:], op=alu.add)
                z2 = rpool.tile([128, 512], bf16, tag="z2")
                nc.vector.tensor_scalar(out=z2[:], in0=a2[:], scalar1=0.0,
                                        scalar2=0.0, op0=alu.max, op1=alu.add,
                                        accum_out=P[:, 16 + r:17 + r])
                if r in (1, 3, 5, 7):
                    ce_chunk(r // 2)

            nc.vector.tensor_copy(P[:, 24:40], cps[:, 0:16])
            nc.sync.dma_start(parts[:, :], P[:])
    nc.finalize()
    return nc


def _run_spmd(key, build_fn, in_maps):
    import os
    import time
    from concourse.bass_utils import run_bass_kernel_spmd
    if key not in _CACHE:
        _CACHE[key] = build_fn()
    nc = _CACHE[key]
    trace = bool(int(os.environ.get("KERNEL_TRACE", "0")))
    t0 = time.perf_counter()
    res = run_bass_kernel_spmd(nc, in_maps, core_ids=list(range(N_CORES)),
                               trace=trace)
    LAST_EXEC_NS[key] = (res.exec_time_ns, time.perf_counter() - t0)
    return res.results


def _dilate(m):
    """Binary box dilation, radius R, separable along axes 1..3 of (B,D,H,W)."""
    x = m.astype(np.int32)
    for ax in (1, 2, 3):
        c = np.cumsum(x, axis=ax, dtype=np.int32)
        n = x.shape[ax]
        hi = np.take(c, np.minimum(np.arange(n) + R, n - 1), axis=ax)
        lo_idx = np.arange(n) - R - 1
        lo = np.take(c, np.maximum(lo_idx, 0), axis=ax)
        shape = [1, 1, 1, 1]
        shape[ax] = n
        valid = (lo_idx >= 0).astype(np.int32).reshape(shape)
        x = hi - lo * valid
    return x > 0


def _to_cos_layout(flat):
    """[NV] flat -> [128, COLS] matching the PE/PSUM voxel layout.

    v = 65536*r + 1024*tau + 8*m + n  lives at  [m, 512*r + 8*tau + n].
    """
    return np.ascontiguousarray(
        flat.reshape(NR, TPR, 128, 8).transpose(2, 0, 1, 3).reshape(128, COLS))


def _from_cos_layout(arr):
    """[128, COLS] device layout -> [NV] flat."""
    return np.ascontiguousarray(
        arr.reshape(128, NR, TPR, 8).transpose(1, 2, 0, 3)).reshape(NV)


def _to_stationary(fcore):
    """[16, NV] f32 channel-major -> [128, NT*128] interleaved stationary.

    out[8c+j, 128t+m] = f[c, 1024t + 8m + j].
    """
    return np.ascontiguousarray(
        fcore.reshape(CF, NT, 128, 8).transpose(0, 3, 1, 2).reshape(128, NT * 128))


def kernel(feature, net_output, target):
    import ml_dtypes
    bf16 = ml_dtypes.bfloat16
    f8 = ml_dtypes.float8_e4m3
    feature = np.ascontiguousarray(np.asarray(feature, dtype=np.float32))
    net_output = np.ascontiguousarray(np.asarray(net_output, dtype=np.float32))
    t3 = np.asarray(target)[:, 0]                      # (B,D,H,W) int32
    pos = t3 == 1
    neg = t3 == 0
    easy = _dilate(pos) & ~pos

    # ---- host: exact std_n (f64 combine of per-batch f32 BLAS matvecs) ----
    possum = np.zeros(CF, np.float64)
    for b in range(B):
        possum += (feature[b].reshape(CF, -1)
                   @ pos[b].reshape(-1).astype(np.float32)).astype(np.float64)
    cnt_pos = float(pos.sum())
    std = possum / max(cnt_pos, 1.0)
    if cnt_pos <= 0:
        std = np.zeros_like(std)
    stdn = std / max(np.linalg.norm(std), 1e-12)

    # ---- host: exact CE linear terms ----
    netf = net_output.reshape(B, CLS, -1)
    sum_x0 = float(netf[:, 0].sum(dtype=np.float64))
    sum_xt = float(np.take_along_axis(
        netf, t3.reshape(B, 1, -1).astype(np.int64), axis=1).sum(dtype=np.float64))

    # ---- selector: cols 0..7 = std_n block-diag, 8..15 = ones block-diag ----
    selm = np.zeros((128, 16), np.float32)
    for c in range(CF):
        for j in range(8):
            selm[8 * c + j, j] = stdn[c]
            selm[8 * c + j, 8 + j] = 1.0
    selm = selm.astype(bf16)

    in_maps = []
    for ci in range(N_CORES):
        b = ci // (N_CORES // B)
        d0 = (ci % (N_CORES // B)) * D_PER_CORE
        fcore = feature[b, :, d0:d0 + D_PER_CORE].reshape(CF, NV)
        fst = _to_stationary(fcore)
        tsh = t3[b, d0:d0 + D_PER_CORE].reshape(NV)
        psh = pos[b, d0:d0 + D_PER_CORE].reshape(NV)
        esh = easy[b, d0:d0 + D_PER_CORE].reshape(NV)
        nsh = net_output[b, :, d0:d0 + D_PER_CORE].reshape(CLS, NV)
        in_maps.append({
            "feat": fst.astype(f8),
            "fsq": (fst.astype(np.float64) ** 2).astype(f8),
            "dd": np.stack([nsh[1] - nsh[0], nsh[2] - nsh[0]]).astype(bf16),
            "y1": _to_cos_layout((tsh == 1).astype(bf16)).reshape(NV),
            "y2": _to_cos_layout((tsh == 2).astype(bf16)).reshape(NV),
            "lea": _to_cos_layout(np.where(esh, np.float32(0.0),
                                           np.float32(NEG_INF)).astype(bf16)),
            "sel": selm,
        })

    results = _run_spmd("fused", build_fused, in_maps)

    # ---- combine partials (f64) ----
    Psum = np.zeros(64, np.float64)
    cos_full = np.empty((B, D_PER_CORE * (N_CORES // B), S, S), np.float32)
    for ci, res in enumerate(results):
        b = ci // (N_CORES // B)
        d0 = (ci % (N_CORES // B)) * D_PER_CORE
        Psum += res["parts"].astype(np.float64).sum(axis=0)
        cos_full[b, d0:d0 + D_PER_CORE] = _from_cos_layout(
            res["cos"].astype(np.float32)).reshape(D_PER_CORE, S, S)

    sum_p1 = Psum[24:40:4].sum()
    sum_p2 = Psum[25:40:4].sum()
    tp1 = Psum[26:40:4].sum()
    tp2 = Psum[27:40:4].sum()
    sum_lns_rel = Psum[40:44].sum()
    poscos = Psum[8:16].sum() - POS_SHIFT * cnt_pos
    easysum = Psum[16:24].sum()

    ce = -(sum_xt - sum_x0 - sum_lns_rel) / NVOX

    cnt1 = float((t3 == 1).sum())
    cnt2 = float((t3 == 2).sum())
    tp = np.array([0.0, tp1, tp2])
    sump = np.array([0.0, sum_p1, sum_p2])
    cntk = np.array([0.0, cnt1, cnt2])
    fp = sump - tp
    fn = cntk - tp
    dc = (2.0 * tp + SMOOTH) / np.maximum(2.0 * tp + fp + fn + SMOOTH, 1e-8)
    dc_loss = -dc[1:].mean()

    pos_loss = (cnt_pos - poscos) / max(cnt_pos, 1.0) if cnt_pos > 0 else 0.0
    easy_cnt = float(easy.sum())
    mis_loss = easysum / max(easy_cnt, 1.0) if easy_cnt > 0 else 0.0

    # ---- host: global top-250 (wide candidate set, exact f64 re-rank) ----
    sims = np.where(neg, cos_full, np.float32(-1e30)).ravel()
    ci_idx = np.argpartition(sims, sims.size - CAND)[-CAND:]
    ci_idx = ci_idx[sims[ci_idx] > -1e29]
    fmat = np.moveaxis(feature, 1, -1).reshape(-1, CF)
    fc = fmat[ci_idx].astype(np.float64)
    nrm = np.maximum(np.linalg.norm(fc, axis=1), 1e-12)
    exact = (fc @ stdn) / nrm
    order = np.argsort(-exact, kind="stable")[:TOP_N]
    keep = ci_idx[order]
    hi = np.zeros(sims.shape, bool)
    hi[keep] = True
    final_neg = _dilate(hi.reshape(B, S, S, S)) & ~pos
    fn_cnt = float(final_neg.sum())
    if fn_cnt > 0:
        neg_loss = float(
            np.maximum(cos_full[final_neg], 0.0).astype(np.float64).sum()
        ) / max(fn_cnt, 1.0)
    else:
        neg_loss = 0.0

    fr = pos_loss + mis_loss + neg_loss
    total = WEIGHT_CE * ce + WEIGHT_DICE * dc_loss + FR_WEIGHT * fr
    return np.asarray(total, dtype=np.float32)


# revision 25
# speedup vs baseline: 1.1317x; 1.0091x over previous
# Trainium2 Bass kernel for nn_DC_and_CE_loss (CE + Dice + feature-regularization).
#
# Single fused device pass (vs the old 2-pass design). Key ideas:
#
# * std_n (the normalized mean-positive feature direction) only depends on
#   `feature` and `target`, so the host computes it exactly (f64) before
#   launch — this removes the pass-1 -> pass-2 device dependency entirely.
# * The per-voxel channel contractions (dot = f . std_n and ss = sum_c f_c^2)
#   run on the otherwise-idle TensorEngine: the feature shard is shipped in a
#   "stationary" interleaved layout [128 = 16ch x 8slot, 128 vox] so each
#   [128,128] fp8 weight tile + one tiny [128,8] selector matmul produces
#   1024 voxel dots as full-width [128, 512] PSUM tiles (FWL loads fp8
#   weights 4/cycle; no PSUM evacuation needed).
# * 1/||f|| = exp(-0.5 * ln(ss + 1e-24)) on ACT (Rsqrt/Reciprocal are banned;
#   Ln/Exp share one table set with the CE exps -> zero table swaps).
# * All masked sums use shifted-relu / shifted-exp encodings so they run as
#   cheap ACT/DVE ops with f32 accum_out instead of the slow (2.8us)
#   scalar_tensor_tensor+accum chains:
#     sum_pos cos       = sum relu(cos + (pos ? 2 : -1e30)) - 2*cnt_pos
#     sum_easy relu cos = sum relu(cos + (easy ? 0 : -1e30))
#     sum p_k           = sum exp(x_k - lns)
#     sum_k p_k y_k     = e^-16 * sum exp(x_k - lns + (y_k ? 16 : -1e30))
#   and CE uses lns = x0 + ln1p(e^{x1-x0} + e^{x2-x0}) so only ln1p's sum is
#   needed from the device (sum x_t and sum x0 are exact host reductions).
# * GPSIMD does nothing (is_equal there costs 14.5us/tile).
# * feature + feature^2 ship as fp8e4 (halves HBM traffic); the top-250
#   selection is protected by a wide candidate set (8192) re-ranked exactly
#   on host in f64 — validated: worst true-top-250 noisy rank = 427.
#
# Host handles (as in the original baseline): masks/dilation from target,
# the global top-k + final_neg dilation, and the tiny f64 combines.

import numpy as np

B, CF, CLS, S = 2, 16, 3, 128
N_CORES = 8
D_PER_CORE = S // (N_CORES // B)       # 32
NV = D_PER_CORE * S * S                # 524288 voxels per core
NVOX = B * S * S * S                   # 4194304
NT = NV // 1024                        # 512 stationary tiles per core
NR = 8                                 # FR rounds per core
TPR = NT // NR                         # 64 tiles per round
COLS = NV // 128                       # 4096
R = 10
TOP_N = 250
SMOOTH = 1e-5
WEIGHT_CE = 1.0
WEIGHT_DICE = 1.0
FR_WEIGHT = 5.0
SHIFT = 16.0                           # exp-mask shift (e^SHIFT rescaled on host)
NEG_INF = -1e30
POS_SHIFT = 2.0
CAND = 16384

_CACHE = {}
LAST_EXEC_NS = {}


def _pin_act_table(mybir, arch):
    """Steer the act-table chooser to the one set that serves BOTH Exp and
    Ln (natural_log_exp_and_others). The default chooser picks the first
    set per function (exp_and_others / natural_log), which thrashes
    ACT_TABLE_LOAD (1.28us each) on every Ln<->Exp alternation — 15 loads
    per kernel. get_activation_tables is functools.cache'd, so in-place
    mutation of the returned sets is seen by insert_act_table_loads."""
    import concourse.hw_specs as hw_specs
    tables = hw_specs.get_activation_tables(arch)
    both = {mybir.ActivationFunctionType.Exp, mybir.ActivationFunctionType.Ln}
    for name, funcs in tables.items():
        if name != "natural_log_exp_and_others":
            funcs -= both


def build_fused():
    import concourse.bacc as bacc
    import concourse.mybir as mybir
    from concourse.tile import TileContext

    f32 = mybir.dt.float32
    bf16 = mybir.dt.bfloat16
    f8 = mybir.dt.float8e4
    alu = mybir.AluOpType
    act = mybir.ActivationFunctionType

    nc = bacc.Bacc("TRN2", debug=False)
    _pin_act_table(mybir, nc.m.arch)
    feat = nc.dram_tensor("feat", [128, NT * 128], f8, kind="ExternalInput").ap()
    fsq = nc.dram_tensor("fsq", [128, NT * 128], f8, kind="ExternalInput").ap()
    dd = nc.dram_tensor("dd", [2, NV], bf16, kind="ExternalInput").ap()
    y1 = nc.dram_tensor("y1", [NV], bf16, kind="ExternalInput").ap()
    y2 = nc.dram_tensor("y2", [NV], bf16, kind="ExternalInput").ap()
    lea = nc.dram_tensor("lea", [128, COLS], bf16, kind="ExternalInput").ap()
    sel = nc.dram_tensor("sel", [128, 16], bf16, kind="ExternalInput").ap()
    cos = nc.dram_tensor("cos", [128, COLS], bf16, kind="ExternalOutput").ap()
    parts = nc.dram_tensor("parts", [128, 64], f32, kind="ExternalOutput").ap()

    with TileContext(nc) as tc, \
         nc.allow_low_precision(reason="bf16/fp8 chains; all sums accumulate f32"):
        with tc.tile_pool(name="const", bufs=1) as cpool, \
             tc.tile_pool(name="ce", bufs=2) as cepool, \
             tc.tile_pool(name="fp", bufs=3) as fpool, \
             tc.tile_pool(name="qp", bufs=3) as qpool, \
             tc.tile_pool(name="mp", bufs=3) as mpool, \
             tc.tile_pool(name="rp", bufs=3) as rpool, \
             tc.tile_pool(name="ps", bufs=3, space="PSUM") as pspool, \
             tc.tile_pool(name="cp", bufs=1, space="PSUM") as cppool:
            P = cpool.tile([128, 64], f32, tag="P")
            nc.vector.memset(P[:], 0.0)
            selt = cpool.tile([128, 16], bf16, tag="sel")
            nc.sync.dma_start(selt[:], sel[:, :])
            bias24 = cpool.tile([128, 1], f32, tag="bias24")
            nc.vector.memset(bias24[:], 1e-24)
            nhalf = cpool.tile([128, 1], f32, tag="nhalf")
            nc.vector.memset(nhalf[:], -0.5)

            nm1 = cpool.tile([128, 1], f32, tag="nm1")
            nc.vector.memset(nm1[:], -1.0)
            ones1 = cpool.tile([128, 1], bf16, tag="ones1")
            nc.vector.memset(ones1[:], 1.0)

            # software-pipelined round DMAs: round r+1's inputs issue
            # while round r computes (pools bufs=2 keep exactly 2 live).
            rt = [None] * NR

            def dma_round(r):
                # quarter-granularity DMAs: the first matmuls of the round
                # only wait on the first 2048-col quarter, not the full chunk
                fc = fpool.tile([128, TPR * 128], f8, tag="fc")
                qc = qpool.tile([128, TPR * 128], f8, tag="qc")
                Q = TPR * 128 // 4
                base = r * TPR * 128
                for q in range(4):
                    nc.sync.dma_start(fc[:, q * Q:(q + 1) * Q],
                                      feat[:, base + q * Q:base + (q + 1) * Q])
                for q in range(4):
                    nc.sync.dma_start(qc[:, q * Q:(q + 1) * Q],
                                      fsq[:, base + q * Q:base + (q + 1) * Q])
                le = mpool.tile([128, 512], bf16, tag="le")
                nc.sync.dma_start(le[:], lea[:, r * 512:(r + 1) * 512])
                rt[r] = (fc, qc, le)

            dma_round(0)

            # ---------------- CE inputs (whole tiles, 1 MiB each) ----------------
            m1 = cepool.tile([128, COLS], bf16, tag="m1")
            nc.sync.dma_start(m1[:], y1[:].rearrange("(p f) -> p f", p=128))
            m2 = cepool.tile([128, COLS], bf16, tag="m2")
            nc.sync.dma_start(m2[:], y2[:].rearrange("(p f) -> p f", p=128))
            dt1 = cepool.tile([128, COLS], bf16, tag="dt1")
            nc.sync.dma_start(dt1[:], dd[0, :].rearrange("(p f) -> p f", p=128))
            dt2 = cepool.tile([128, COLS], bf16, tag="dt2")
            nc.sync.dma_start(dt2[:], dd[1, :].rearrange("(p f) -> p f", p=128))
            cps = cppool.tile([128, 16], f32, tag="cps")

            NCE = 4
            CW = COLS // NCE

            def ce_chunk(k):
                """CE/dice partial chunk k over cols [CW*k, CW*(k+1))."""
                sl = slice(CW * k, CW * (k + 1))
                e1 = cepool.tile([128, CW], bf16, tag="e1")
                nc.scalar.activation(e1[:], dt1[:, sl], act.Exp)
                e2 = cepool.tile([128, CW], bf16, tag="e2")
                nc.scalar.activation(e2[:], dt2[:, sl], act.Exp)
                sm = cepool.tile([128, CW], bf16, tag="sm")
                nc.vector.tensor_tensor(out=sm[:], in0=e1[:], in1=e2[:], op=alu.add)
                lr = cepool.tile([128, CW], bf16, tag="lr")
                nc.scalar.activation(lr[:], sm[:], act.Ln, bias=1.0,
                                     accum_out=P[:, 40 + k:41 + k])
                wt = cepool.tile([128, CW], bf16, tag="wt")
                nc.scalar.activation(wt[:], lr[:], act.Exp, scale=nm1[:, 0:1])
                p1 = cepool.tile([128, CW], bf16, tag="p1")
                nc.vector.tensor_tensor(out=p1[:], in0=e1[:], in1=wt[:], op=alu.mult)
                p2 = cepool.tile([128, CW], bf16, tag="p2")
                nc.vector.tensor_tensor(out=p2[:], in0=e2[:], in1=wt[:], op=alu.mult)
                t1 = cepool.tile([128, CW], bf16, tag="t1")
                nc.vector.tensor_tensor(out=t1[:], in0=p1[:], in1=m1[:, sl], op=alu.mult)
                t2 = cepool.tile([128, CW], bf16, tag="t2")
                nc.vector.tensor_tensor(out=t2[:], in0=p2[:], in1=m2[:, sl], op=alu.mult)
                for i, src in enumerate((p1, p2, t1, t2)):
                    for c in range(CW // 128):
                        nc.tensor.matmul(cps[:, 4 * k + i:4 * k + i + 1],
                                         src[:, 128 * c:128 * c + 128],
                                         ones1[:, 0:1],
                                         start=(c == 0), stop=(c == CW // 128 - 1))

            # ---------------- FR rounds ----------------
            for r in range(NR):
                if r + 1 < NR:
                    dma_round(r + 1)
                fc, qc, le = rt[r]

                # ss matmuls first so ACT's Ln overlaps the dot matmuls
                pd = pspool.tile([128, 512], f32, tag="pd")
                ps = pspool.tile([128, 512], f32, tag="ps")
                for t in range(TPR):
                    nc.tensor.matmul(ps[:, 8 * t:8 * t + 8],
                                     qc[:, 128 * t:128 * t + 128],
                                     selt[:, 8:16], start=True, stop=True)
                for t in range(TPR):
                    nc.tensor.matmul(pd[:, 8 * t:8 * t + 8],
                                     fc[:, 128 * t:128 * t + 128],
                                     selt[:, 0:8], start=True, stop=True)

                # last round: process the chain in column-quarters so the
                # final drain after the last matmul is ~4x shorter
                nq = 4 if r == NR - 1 else 1
                qw = 512 // nq
                lnt = rpool.tile([128, 512], f32, tag="lnt")
                rv = rpool.tile([128, 512], bf16, tag="rv")
                co = rpool.tile([128, 512], bf16, tag="co")
                sp = rpool.tile([128, 512], bf16, tag="sp")
                a1 = rpool.tile([128, 512], bf16, tag="a1")
                z1 = rpool.tile([128, 512], bf16, tag="z1")
                a2 = rpool.tile([128, 512], bf16, tag="a2")
                z2 = rpool.tile([128, 512], bf16, tag="z2")
                for q in range(nq):
                    cq = slice(q * qw, (q + 1) * qw)
                    pc1 = P[:, 8 + r:9 + r] if nq == 1 else P[:, 44 + q:45 + q]
                    pc2 = P[:, 16 + r:17 + r] if nq == 1 else P[:, 48 + q:49 + q]
                    nc.scalar.activation(lnt[:, cq], ps[:, cq], act.Ln,
                                         bias=bias24[:, 0:1])
                    nc.scalar.activation(rv[:, cq], lnt[:, cq], act.Exp,
                                         scale=nhalf[:, 0:1])
                    nc.vector.tensor_tensor(out=co[:, cq], in0=pd[:, cq],
                                            in1=rv[:, cq], op=alu.mult)
                    nc.sync.dma_start(cos[:, r * 512 + q * qw:r * 512 + (q + 1) * qw],
                                      co[:, cq])
                    nc.vector.tensor_scalar(
                        out=sp[:, cq], in0=m1[:, r * 512 + q * qw:r * 512 + (q + 1) * qw],
                        scalar1=242.0, scalar2=-240.0, op0=alu.mult, op1=alu.add)
                    nc.vector.tensor_tensor(out=a1[:, cq], in0=co[:, cq],
                                            in1=sp[:, cq], op=alu.add)
                    nc.vector.tensor_scalar(out=z1[:, cq], in0=a1[:, cq], scalar1=0.0,
                                            scalar2=0.0, op0=alu.max, op1=alu.add,
                                            accum_out=pc1)
                    nc.vector.tensor_tensor(out=a2[:, cq], in0=co[:, cq],
                                            in1=le[:, cq], op=alu.add)
                    nc.vector.tensor_scalar(out=z2[:, cq], in0=a2[:, cq], scalar1=0.0,
                                            scalar2=0.0, op0=alu.max, op1=alu.add,
                                            accum_out=pc2)
                if r in (1, 3, 5, 6):
                    ce_chunk((1, 3, 5, 6).index(r))

            nc.vector.tensor_copy(P[:, 24:40], cps[:, 0:16])
            nc.sync.dma_start(parts[:, :], P[:])
    nc.finalize()
    return nc


def _run_spmd(key, build_fn, in_maps):
    import os
    import time
    from concourse.bass_utils import run_bass_kernel_spmd
    if key not in _CACHE:
        _CACHE[key] = build_fn()
    nc = _CACHE[key]
    trace = bool(int(os.environ.get("KERNEL_TRACE", "0")))
    t0 = time.perf_counter()
    res = run_bass_kernel_spmd(nc, in_maps, core_ids=list(range(N_CORES)),
                               trace=trace)
    LAST_EXEC_NS[key] = (res.exec_time_ns, time.perf_counter() - t0)
    return res.results


def _dilate(m):
    """Binary box dilation, radius R, separable along axes 1..3 of (B,D,H,W)."""
    x = m.astype(np.int32)
    for ax in (1, 2, 3):
        c = np.cumsum(x, axis=ax, dtype=np.int32)
        n = x.shape[ax]
        hi = np.take(c, np.minimum(np.arange(n) + R, n - 1), axis=ax)
        lo_idx = np.arange(n) - R - 1
        lo = np.take(c, np.maximum(lo_idx, 0), axis=ax)
        shape = [1, 1, 1, 1]
        shape[ax] = n
        valid = (lo_idx >= 0).astype(np.int32).reshape(shape)
        x = hi - lo * valid
    return x > 0


def _to_cos_layout(flat):
    """[NV] flat -> [128, COLS] matching the PE/PSUM voxel layout.

    v = 65536*r + 1024*tau + 8*m + n  lives at  [m, 512*r + 8*tau + n].
    """
    return np.ascontiguousarray(
        flat.reshape(NR, TPR, 128, 8).transpose(2, 0, 1, 3).reshape(128, COLS))


def _from_cos_layout(arr):
    """[128, COLS] device layout -> [NV] flat."""
    return np.ascontiguousarray(
        arr.reshape(128, NR, TPR, 8).transpose(1, 2, 0, 3)).reshape(NV)


def _to_stationary(fcore):
    """[16, NV] f32 channel-major -> [128, NT*128] interleaved stationary.

    out[8c+j, 128t+m] = f[c, 1024t + 8m + j].
    """
    return np.ascontiguousarray(
        fcore.reshape(CF, NT, 128, 8).transpose(0, 3, 1, 2).reshape(128, NT * 128))


def kernel(feature, net_output, target):
    import ml_dtypes
    bf16 = ml_dtypes.bfloat16
    f8 = ml_dtypes.float8_e4m3
    feature = np.ascontiguousarray(np.asarray(feature, dtype=np.float32))
    net_output = np.ascontiguousarray(np.asarray(net_output, dtype=np.float32))
    t3 = np.asarray(target)[:, 0]                      # (B,D,H,W) int32
    pos = t3 == 1
    neg = t3 == 0
    easy = _dilate(pos) & ~pos

    # ---- host: exact std_n (f64 combine of per-batch f32 BLAS matvecs) ----
    possum = np.zeros(CF, np.float64)
    for b in range(B):
        possum += (feature[b].reshape(CF, -1)
                   @ pos[b].reshape(-1).astype(np.float32)).astype(np.float64)
    cnt_pos = float(pos.sum())
    std = possum / max(cnt_pos, 1.0)
    if cnt_pos <= 0:
        std = np.zeros_like(std)
    stdn = std / max(np.linalg.norm(std), 1e-12)

    # ---- host: exact CE linear terms ----
    netf = net_output.reshape(B, CLS, -1)
    sum_x0 = float(netf[:, 0].sum(dtype=np.float64))
    sum_xt = float(np.take_along_axis(
        netf, t3.reshape(B, 1, -1).astype(np.int64), axis=1).sum(dtype=np.float64))

    # ---- selector: cols 0..7 = std_n block-diag, 8..15 = ones block-diag ----
    selm = np.zeros((128, 16), np.float32)
    for c in range(CF):
        for j in range(8):
            selm[8 * c + j, j] = stdn[c]
            selm[8 * c + j, 8 + j] = 1.0
    selm = selm.astype(bf16)

    in_maps = []
    for ci in range(N_CORES):
        b = ci // (N_CORES // B)
        d0 = (ci % (N_CORES // B)) * D_PER_CORE
        fcore = feature[b, :, d0:d0 + D_PER_CORE].reshape(CF, NV)
        fst = _to_stationary(fcore)
        tsh = t3[b, d0:d0 + D_PER_CORE].reshape(NV)
        psh = pos[b, d0:d0 + D_PER_CORE].reshape(NV)
        esh = easy[b, d0:d0 + D_PER_CORE].reshape(NV)
        nsh = net_output[b, :, d0:d0 + D_PER_CORE].reshape(CLS, NV)
        in_maps.append({
            "feat": fst.astype(f8),
            "fsq": (fst.astype(np.float64) ** 2).astype(f8),
            "dd": np.stack([nsh[1] - nsh[0], nsh[2] - nsh[0]]).astype(bf16),
            "y1": _to_cos_layout((tsh == 1).astype(bf16)).reshape(NV),
            "y2": _to_cos_layout((tsh == 2).astype(bf16)).reshape(NV),
            "lea": _to_cos_layout(np.where(esh, np.float32(0.0),
                                           np.float32(NEG_INF)).astype(bf16)),
            "sel": selm,
        })

    results = _run_spmd("fused", build_fused, in_maps)

    # ---- combine partials (f64) ----
    Psum = np.zeros(64, np.float64)
    cos_full = np.empty((B, D_PER_CORE * (N_CORES // B), S, S), np.float32)
    for ci, res in enumerate(results):
        b = ci // (N_CORES // B)
        d0 = (ci % (N_CORES // B)) * D_PER_CORE
        Psum += res["parts"].astype(np.float64).sum(axis=0)
        cos_full[b, d0:d0 + D_PER_CORE] = _from_cos_layout(
            res["cos"].astype(np.float32)).reshape(D_PER_CORE, S, S)

    sum_p1 = Psum[24:40:4].sum()
    sum_p2 = Psum[25:40:4].sum()
    tp1 = Psum[26:40:4].sum()
    tp2 = Psum[27:40:4].sum()
    sum_lns_rel = Psum[40:44].sum()
    poscos = Psum[8:15].sum() + Psum[44:48].sum() - POS_SHIFT * cnt_pos
    easysum = Psum[16:23].sum() + Psum[48:52].sum()

    ce = -(sum_xt - sum_x0 - sum_lns_rel) / NVOX

    cnt1 = float((t3 == 1).sum())
    cnt2 = float((t3 == 2).sum())
    tp = np.array([0.0, tp1, tp2])
    sump = np.array([0.0, sum_p1, sum_p2])
    cntk = np.array([0.0, cnt1, cnt2])
    fp = sump - tp
    fn = cntk - tp
    dc = (2.0 * tp + SMOOTH) / np.maximum(2.0 * tp + fp + fn + SMOOTH, 1e-8)
    dc_loss = -dc[1:].mean()

    pos_loss = (cnt_pos - poscos) / max(cnt_pos, 1.0) if cnt_pos > 0 else 0.0
    easy_cnt = float(easy.sum())
    mis_loss = easysum / max(easy_cnt, 1.0) if easy_cnt > 0 else 0.0

    # ---- host: global top-250 (wide candidate set, exact f64 re-rank) ----
    sims = np.where(neg, cos_full, np.float32(-1e30)).ravel()
    ci_idx = np.argpartition(sims, sims.size - CAND)[-CAND:]
    ci_idx = ci_idx[sims[ci_idx] > -1e29]
    fmat = np.moveaxis(feature, 1, -1).reshape(-1, CF)
    fc = fmat[ci_idx].astype(np.float64)
    nrm = np.maximum(np.linalg.norm(fc, axis=1), 1e-12)
    exact = (fc @ stdn) / nrm
    order = np.argsort(-exact, kind="stable")[:TOP_N]
    keep = ci_idx[order]
    hi = np.zeros(sims.shape, bool)
    hi[keep] = True
    final_neg = _dilate(hi.reshape(B, S, S, S)) & ~pos
    fn_cnt = float(final_neg.sum())
    if fn_cnt > 0:
        neg_loss = float(
            np.maximum(cos_full[final_neg], 0.0).astype(np.float64).sum()
        ) / max(fn_cnt, 1.0)
    else:
        neg_loss = 0.0

    fr = pos_loss + mis_loss + neg_loss
    total = WEIGHT_CE * ce + WEIGHT_DICE * dc_loss + FR_WEIGHT * fr
    return np.asarray(total, dtype=np.float32)
